# revision 11
# baseline (speedup 1.0000x reference)
"""Causal self-attention (B=2, T=2048, C=1024, H=16) on 8 TRN2 NeuronCores.

Sharding: core c -> batch b = c//4, head group hg = c%4 (4 heads/core).
Each core computes QKV for its 4 heads (column-parallel), causal attention,
and a row-parallel partial output projection [T, C] in fp16. The host sums
the 4 partials per batch and adds the analytically-folded biases.

Device layouts (chosen so no on-chip transposes are ever needed):
  xt   [C=1024, T=2048] bf16   x[b] transposed (host-prepped)
  Q^T  [128, pair, T]   bf16   head pair packed on partitions (0-63 / 64-127)
  K^T  same
  vaug [128, tj, 4*66]  bf16   per head: cols0-63 = V[tj block], col64 = ones
  S^T  [k=128, q<=1024] psum   one K=64 matmul per (h, key chunk) covering the
                               contiguous query span of BOTH groups of a pair
  P^T = exp(S^T)        bf16   one ACT instr per chunk (no max subtraction)
  O^T  [65, 1024] psum  rows 0-63 = (P@V)^T, row 64 = softmax denominator l,
                        both query groups side by side (single accum group)
  yt   [128(h,d), T]    bf16   normalized attention output, feeds proj lhsT

Attention processes query groups in pairs (0,1) and (2,3): per (head, key
chunk) a single wide matmul covers both groups' queries, so S needs one
LDWEIGHTS per chunk instead of one per (chunk, group); same for PV and exp.
PV trails S by one chunk so the ACT exp stays off the PE critical path.
The softmax denominator reciprocal is broadcast across partitions with a
ones[1,64] PE matmul instead of a DRAM round-trip.
"""

import sys

if "/opt/trn_rl_repo" not in sys.path:
    sys.path.insert(0, "/opt/trn_rl_repo")

import numpy as np
import ml_dtypes
from contextlib import ExitStack

import concourse.bass as bass
import concourse.mybir as mybir
import concourse.tile as tile
from concourse import bacc, bass_utils
from concourse.bass import ds, ts


BF = mybir.dt.bfloat16
F16 = mybir.dt.float16
F32 = mybir.dt.float32

B, T, C = 2, 2048, 1024
H, DK = 16, 64
P = 128
KC = C // P          # 8 contraction chunks over C
NTG = T // 512       # 4 t-groups of 512
NTJ = T // 128       # 16 t-chunks of 128
HPC = 4              # heads per core
VS = 66              # vaug per-head stride (cols 0-63 V, 64 ones, 65 pad)

NEG = -30000.0

# module-level knobs for test harness
TRACE = False
TRACE_KWARGS = {}
LAST_RESULTS = None


def _emit(ctx, tc, aps):
    nc = tc.nc
    xt, wq, wk, wv, bq, bk, wp, mask, out = (
        aps["xt"], aps["wq"], aps["wk"], aps["wv"], aps["bq"], aps["bk"],
        aps["wp"], aps["mask"], aps["out"],
    )

    consts = ctx.enter_context(tc.tile_pool(name="consts", bufs=1))
    bigs = ctx.enter_context(tc.tile_pool(name="bigs", bufs=1))
    temps = ctx.enter_context(tc.tile_pool(name="temps", bufs=4))
    ppool = ctx.enter_context(tc.tile_pool(name="ppool", bufs=4))
    psum = ctx.enter_context(tc.tile_pool(name="psum", bufs=1, space="PSUM"))
    dpool = ctx.enter_context(tc.tile_pool(name="dpool", bufs=2, space="DRAM"))

    # ---- load inputs to SBUF, issued in consumption order. Both queues carry
    # the first two x t-groups split by k-chunk so the first QKV k-loop can
    # stream chunks as they land; everything not needed before ~20us (mask,
    # tg2/tg3, wvs, wps) is issued after them so the round-robin DMA engines
    # don't dilute the critical transfers. ----
    wqs = consts.tile([P, KC, 2 * P], BF)
    nc.sync.dma_start(out=wqs, in_=wq)
    wks = consts.tile([P, KC, 2 * P], BF)
    nc.gpsimd.dma_start(out=wks, in_=wk)

    xts = bigs.tile([P, KC, T], BF)
    for k0, k1 in ((0, 2), (2, 5), (5, 8)):
        nc.sync.dma_start(out=xts[:, k0:k1, 0:512], in_=xt[0, :, k0:k1, :])
        nc.gpsimd.dma_start(out=xts[:, k0:k1, 512:1024], in_=xt[1, :, k0:k1, :])

    bqs = consts.tile([P, 2], F32)
    nc.sync.dma_start(out=bqs, in_=bq.rearrange("(m p) -> p m", p=P))
    bks = consts.tile([P, 2], F32)
    nc.sync.dma_start(out=bks, in_=bk.rearrange("(m p) -> p m", p=P))
    maskt = consts.tile([P, P], F32)
    nc.sync.dma_start(out=maskt, in_=mask)
    wvs = consts.tile([P, KC, 2 * P], BF)
    nc.gpsimd.dma_start(out=wvs, in_=wv)
    nc.sync.dma_start(out=xts[:, :, ts(2, 512)], in_=xt[2])
    nc.gpsimd.dma_start(out=xts[:, :, ts(3, 512)], in_=xt[3])
    wps = consts.tile([P, 2, C], BF)
    nc.sync.dma_start(out=wps, in_=wp)

    ones164 = consts.tile([1, DK], BF)
    nc.vector.memset(ones164, 1.0)

    # ---- Q^T / K^T: [128(d pair-packed), pair, T] ----
    qt = bigs.tile([P, 2, T], BF)
    kt = bigs.tile([P, 2, T], BF)

    def emit_qk_pair(m, pg):
        # single 1024-wide matmul covers t-groups 2*pg, 2*pg+1 -> one
        # LDWEIGHTS per contraction chunk
        for wsrc, bsrc, dst in ((wqs, bqs, qt), (wks, bks, kt)):
            pq = psum.tile([P, 2 * 512], F32, tag="s", bufs=2, name="pq")
            for k in range(KC):
                for i in range(2):
                    nc.tensor.matmul(
                        pq[:, ts(i, 512)],
                        lhsT=wsrc[:, k, ts(m, P)],
                        rhs=xts[:, k, ts(2 * pg + i, 512)],
                        start=(k == 0),
                        stop=(k == KC - 1),
                        skip_group_check=True,
                    )
            nc.vector.tensor_add(
                out=dst[:, m, ts(pg, 1024)],
                in0=pq,
                in1=bsrc[:, m : m + 1].to_broadcast([P, 1024]),
            )

    # ---- V -> vaug [128, tj, 4*66] (col DK = ones) ----
    vaug = bigs.tile([P, NTJ, HPC * VS], BF)
    vaug4 = vaug.rearrange("p t (h c) -> p t h c", c=VS)

    def emit_v(g):
        for tj in range(4 * g, 4 * g + 4):
            pv = psum.tile([P, 512], F32, tag="mm", bufs=2, name="pv")
            for k in range(KC):
                nc.tensor.matmul(
                    pv[:, : 2 * P],
                    lhsT=xts[:, k, ts(tj, P)],
                    rhs=wvs[:, k, :],
                    start=(k == 0),
                    stop=(k == KC - 1),
                )
            nc.vector.tensor_copy(
                out=vaug4[:, tj, :, 0:DK],
                in_=pv[:, : 2 * P].rearrange("p (h d) -> p h d", d=DK),
            )

    # ---- attention ----
    yts = [bigs.tile([P, T], BF, name=f"yt{m}") for m in range(2)]

    def halves(begin):
        # split a [begin, 1024) span at the PSUM bank boundary (matmul out
        # is ISA-capped at 512); same-tile halves stay adjacent in the
        # schedule so the second LDWEIGHTS dedupes away
        return [(begin, 512), (512, 1024)] if begin < 512 else [(begin, 1024)]

    def emit_pv(m, h, po, pend, njc):
        j, begin, pt = pend
        for lo, hi in halves(begin):
            # the [*, 512) region belongs to group g0 whose last chunk is
            # njc-5; the [512, *) region accumulates until the final chunk
            stop = (j == njc - 1) if lo >= 512 else (j == njc - 5)
            nc.tensor.matmul(
                po[:, lo:hi],
                lhsT=vaug4[:, j, 2 * m + h, : DK + 1],
                rhs=pt[:, lo:hi],
                start=(j == 0),
                stop=stop,
                skip_group_check=True,
            )

    def emit_attn_pair(m, g0, weave=None):
        """Heads h=0,1 of pair m over query groups g0, g0+1. One S matmul /
        exp / PV accumulation per (h, key chunk) covers the contiguous query
        span of both groups. `weave` thunks are popped at j%8==7 points."""
        g1 = g0 + 1
        njc = 4 * g1 + 4
        for h in range(2):
            po = psum.tile([DK + 1, 2 * 512], F32, tag="o", bufs=1, name="po")
            pend = None
            for j in range(njc):
                jr0, jr1 = j - 4 * g0, j - 4 * g1
                if jr1 >= 0:
                    begin = 512 + 128 * jr1
                elif jr0 >= 0:
                    begin = 128 * jr0
                else:
                    begin = 0
                ps = psum.tile([P, 2 * 512], F32, tag="s", bufs=2, name="ps")
                for lo, hi in halves(begin):
                    nc.tensor.matmul(
                        ps[:, lo:hi],
                        lhsT=kt[h * DK : (h + 1) * DK, m, ts(j, P)],
                        rhs=qt[h * DK : (h + 1) * DK, m,
                               ds(g0 * 512 + lo, hi - lo)],
                        start=True,
                        stop=True,
                        tile_position=(h * DK, 0),
                        skip_group_check=True,
                    )
                # flush previous chunk's PV while ACT runs this chunk's exp
                if pend is not None:
                    emit_pv(m, h, po, pend, njc)
                if 0 <= jr0 < 4:
                    nc.vector.tensor_add(
                        out=ps[:, ds(128 * jr0, P)],
                        in0=ps[:, ds(128 * jr0, P)],
                        in1=maskt,
                    )
                if 0 <= jr1 < 4:
                    nc.vector.tensor_add(
                        out=ps[:, ds(512 + 128 * jr1, P)],
                        in0=ps[:, ds(512 + 128 * jr1, P)],
                        in1=maskt,
                    )
                pt = ppool.tile([P, 2 * 512], BF, tag="p", name="pt")
                nc.scalar.activation(
                    pt[:, begin:1024],
                    ps[:, begin:1024],
                    mybir.ActivationFunctionType.Exp,
                )
                pend = (j, begin, pt)
                if weave and j % 8 == 7:
                    weave.pop(0)()
            emit_pv(m, h, po, pend, njc)
            if weave:
                weave.pop(0)()
            # finalize both groups: copy O^T off PSUM fast (g0 on DVE, g1 on
            # ACT so the copies overlap), reciprocal of the denominator row,
            # broadcast it to 64 partitions with a ones[1,64] PE matmul, then
            # normalize into yt
            for gi, g in enumerate((g0, g1)):
                gsl = ds(gi * 512, 512)
                oc = temps.tile([P, 512], F32, tag="oc", name="oc")
                if gi == 0:
                    nc.vector.tensor_copy(out=oc[: DK + 1, :], in_=po[:, gsl])
                else:
                    nc.scalar.copy(out=oc[: DK + 1, :], in_=po[:, gsl])
                dscr = dpool.tile([512], F32, tag="dscr", name="dscr")
                deng = nc.sync if gi == 0 else nc.gpsimd
                deng.dma_start(out=dscr, in_=oc[DK : DK + 1, :])
                rbl = temps.tile([P, 512], F32, tag="rbl", name="rbl")
                reng = nc.gpsimd if gi == 0 else nc.sync
                reng.dma_start(
                    out=rbl[:DK, :],
                    in_=bass.AP(
                        tensor=dscr.tensor,
                        offset=dscr.offset,
                        ap=[[0, DK]] + list(dscr.ap),
                    ),
                )
                rb = temps.tile([P, 512], F32, tag="rb", name="rb")
                nc.vector.reciprocal_approx_fast(out=rb[:DK, :], in_=rbl[:DK, :])
                stg = temps.tile([P, 512], BF, tag="stg", name="stg")
                nc.vector.tensor_mul(
                    out=stg[:DK, :], in0=oc[:DK, :], in1=rb[:DK, :]
                )
                nc.sync.dma_start(
                    out=yts[m][h * DK : (h + 1) * DK, ts(g, 512)],
                    in_=stg[:DK, :],
                )

    # ---- output projection: fp16 partial [T, C] for one t-chunk of 128 ----
    def emit_proj_tj(tj):
        pp = psum.tile([P, 2 * 512], F32, tag="s", bufs=2, name="pp")
        for kc in range(2):
            for n in range(2):
                nc.tensor.matmul(
                    pp[:, ts(n, 512)],
                    lhsT=yts[kc][:, ts(tj, P)],
                    rhs=wps[:, kc, ts(n, 512)],
                    start=(kc == 0),
                    stop=(kc == 1),
                    skip_group_check=True,
                )
        ostg = temps.tile([P, 2 * 512], F16, tag="ostg", name="ostg")
        nc.vector.tensor_copy(out=ostg, in_=pp)
        eng = nc.sync if tj % 2 == 0 else nc.gpsimd
        eng.dma_start(out=out[ts(tj, P), :], in_=ostg)

    def proj_thunk(tj):
        return lambda: emit_proj_tj(tj)

    # ---- schedule ----
    nc.vector.memset(vaug4[:, :, :, DK : DK + 1], 1.0)
    emit_qk_pair(0, 0)
    emit_v(0)
    emit_v(1)
    emit_attn_pair(0, 0)
    emit_qk_pair(1, 0)
    emit_attn_pair(1, 0)
    emit_qk_pair(0, 1)
    emit_v(2)
    emit_v(3)
    # proj for groups 0/1 is ready now; weave its t-chunks into the pair-2/3
    # attention so the only tail left is proj of groups 2/3
    emit_attn_pair(0, 2, weave=[proj_thunk(tj) for tj in range(0, 4)])
    emit_qk_pair(1, 1)
    emit_attn_pair(1, 2, weave=[proj_thunk(tj) for tj in range(4, 8)])
    for tj in range(8, NTJ):
        emit_proj_tj(tj)


def _dedupe_ldweights(nc):
    """Drop an InstLdweights when the immediately-preceding PE weight load in
    the scheduled stream is byte-identical (only matmuls in between — they
    don't disturb the stationary operand). A duplicate that carries sync_info
    has its waits/updates migrated onto the following matmul (which executes
    strictly later, so waits stay satisfied-before-use and updates fire no
    earlier). Saves ~100ns of serialized PE time per duplicate."""
    removed = 0
    for f in nc.m.functions:
        for bb in f.blocks:
            insts = list(bb.instructions)
            last_sig = None
            to_remove = []
            for pos, inst in enumerate(insts):
                tn = type(inst).__name__
                if tn == "InstLdweights":
                    si = inst.sync_info
                    has_sync = si is not None and (
                        list(si.on_wait) or list(si.on_update)
                    )
                    sig = (
                        str(inst.ins[0]),
                        str(inst.tile_position),
                        str(inst.tile_size),
                        str(inst.perf_mode),
                        str(inst.is_transpose),
                    )
                    if sig == last_sig:
                        if not has_sync:
                            to_remove.append(inst)
                            continue
                        nxt = insts[pos + 1] if pos + 1 < len(insts) else None
                        nsi = getattr(nxt, "sync_info", None) if nxt is not None else None
                        if (
                            nxt is not None
                            and type(nxt).__name__ == "InstMatmult"
                            and nsi is not None
                        ):
                            try:
                                nsi.on_wait = list(si.on_wait) + list(nsi.on_wait)
                                nsi.on_update = list(si.on_update) + list(
                                    nsi.on_update
                                )
                                to_remove.append(inst)
                                continue
                            except Exception:
                                pass
                        last_sig = sig
                        continue
                    last_sig = sig
                elif tn == "InstMatmult":
                    continue
                elif getattr(inst, "engine", None) == mybir.EngineType.PE:
                    last_sig = None
            for inst in to_remove:
                bb.instructions.remove(inst)
                removed += 1
    return removed


_NC_CACHE = None


def build():
    global _NC_CACHE
    if _NC_CACHE is not None:
        return _NC_CACHE
    nc = bacc.Bacc("TRN2", target_bir_lowering=False, debug=False, num_devices=8)
    aps = {
        "xt": nc.dram_tensor("xt", [NTG, P, KC, 512], BF, kind="ExternalInput").ap(),
        "wq": nc.dram_tensor("wq", [P, KC, 2 * P], BF, kind="ExternalInput").ap(),
        "wk": nc.dram_tensor("wk", [P, KC, 2 * P], BF, kind="ExternalInput").ap(),
        "wv": nc.dram_tensor("wv", [P, KC, 2 * P], BF, kind="ExternalInput").ap(),
        "bq": nc.dram_tensor("bq", [2 * P], F32, kind="ExternalInput").ap(),
        "bk": nc.dram_tensor("bk", [2 * P], F32, kind="ExternalInput").ap(),
        "wp": nc.dram_tensor("wp", [P, 2, C], BF, kind="ExternalInput").ap(),
        "mask": nc.dram_tensor("mask", [P, P], F32, kind="ExternalInput").ap(),
        "out": nc.dram_tensor("out", [T, C], F16, kind="ExternalOutput").ap(),
    }
    with tile.TileContext(nc) as tc:
        with ExitStack() as ctx:
            _emit(ctx, tc, aps)
    _dedupe_ldweights(nc)
    nc.compile()
    _NC_CACHE = nc
    return nc


def make_in_maps(x, Wqkv, bqkv, Wproj):
    """Host-side sharding/layout prep. Returns per-core input dicts."""
    bf = ml_dtypes.bfloat16
    scale = np.float32(1.0 / np.sqrt(DK))
    maskv = np.where(
        np.arange(P)[None, :] >= np.arange(P)[:, None], 0.0, NEG
    ).astype(np.float32)
    def lay_w(w):  # [C, n] -> [p, k, n] linear
        n = w.shape[1]
        return np.ascontiguousarray(
            w.reshape(KC, P, n).transpose(1, 0, 2)
        ).astype(bf)

    def lay_x(xb):  # [T, C] -> [tg, p, k, 512] linear
        xt = xb.T  # [C, T]
        return np.ascontiguousarray(
            xt.reshape(KC, P, NTG, 512).transpose(2, 1, 0, 3)
        ).astype(bf)

    xts = [lay_x(x[b]) for b in range(B)]
    in_maps = []
    for c in range(8):
        b, hg = divmod(c, 4)
        lo = hg * HPC * DK
        sl = slice(lo, lo + HPC * DK)
        in_maps.append(
            {
                "xt": xts[b],
                "wq": lay_w(Wqkv[:, 0 * C :][:, sl] * scale),
                "wk": lay_w(Wqkv[:, 1 * C :][:, sl]),
                "wv": lay_w(Wqkv[:, 2 * C :][:, sl]),
                "bq": np.ascontiguousarray(bqkv[0 * C :][sl] * scale).astype(np.float32),
                "bk": np.ascontiguousarray(bqkv[1 * C :][sl]).astype(np.float32),
                "wp": np.ascontiguousarray(
                    Wproj[sl, :].reshape(2, P, C).transpose(1, 0, 2)
                ).astype(bf),
                "mask": maskv,
            }
        )
    return in_maps


def gather(outs, bqkv, Wproj, bproj):
    """Sum per-core fp16 partials per batch; fold V-bias + proj-bias."""
    bv = bqkv[2 * C :].astype(np.float32)
    bp_eff = (bproj.astype(np.float32) + bv @ Wproj.astype(np.float32)).astype(
        np.float32
    )
    y = np.empty((B, T, C), np.float32)
    for b in range(B):
        acc = outs[b * 4 + 0].astype(np.float32)
        for hg in range(1, 4):
            acc += outs[b * 4 + hg].astype(np.float32)
        y[b] = acc + bp_eff[None, :]
    return y


def kernel(x, Wqkv, bqkv, Wproj, bproj):
    global LAST_RESULTS
    x = np.asarray(x, dtype=np.float32)
    Wqkv = np.asarray(Wqkv, dtype=np.float32)
    bqkv = np.asarray(bqkv, dtype=np.float32)
    Wproj = np.asarray(Wproj, dtype=np.float32)
    bproj = np.asarray(bproj, dtype=np.float32)

    nc = build()
    in_maps = make_in_maps(x, Wqkv, bqkv, Wproj)
    try:
        res = bass_utils.run_bass_kernel_spmd(
            nc,
            in_maps,
            core_ids=list(range(8)),
            trace=TRACE,
            **TRACE_KWARGS,
        )
    except Exception:
        if not TRACE:
            raise
        import traceback

        traceback.print_exc()
        print("traced run failed; retrying without trace", file=sys.stderr)
        res = bass_utils.run_bass_kernel_spmd(nc, in_maps, core_ids=list(range(8)))
    LAST_RESULTS = res
    outs = [res.results[c]["out"] for c in range(8)]
    return gather(outs, bqkv, Wproj, bproj)


# revision 12
# speedup vs baseline: 1.1832x; 1.1832x over previous
"""Causal self-attention (B=2, T=2048, C=1024, H=16) on 8 TRN2 NeuronCores.

Sharding: core c -> batch b = c//4, head group hg = c%4 (4 heads/core).
Each core computes QKV for its 4 heads (column-parallel), causal attention,
and a row-parallel partial output projection [T, C]. The host sums the 4
partials per batch and adds the analytically-folded biases.

Device layouts (chosen so no on-chip transposes are ever needed):
  xt   [C=1024, T=2048] bf16   x[b] transposed (host-prepped)
  Q^T  [128, pair, T]   bf16   head pair packed on partitions (0-63 / 64-127)
  K^T  same
  vaug [128, tj, 4*66]  bf16   per head: col0 = ones, cols1-64 = V[tj block]
  S^T  [k=128, q<=512]  psum   row-packed K=64 matmuls, 2 heads concurrent
  P^T = exp(S^T)        bf16   (no max subtraction; scores are ~N(0,1))
  O^T  [65, 512] psum:  row0 = softmax denominator l, rows 1-64 = (P@V)^T
  yt   [128(h,d), T]    bf16   normalized attention output, feeds proj lhsT
"""

import sys

if "/opt/trn_rl_repo" not in sys.path:
    sys.path.insert(0, "/opt/trn_rl_repo")

import numpy as np
import ml_dtypes
from contextlib import ExitStack

import concourse.bass as bass
import concourse.mybir as mybir
import concourse.tile as tile
from concourse import bacc, bass_utils
from concourse.bass import ds, ts


BF = mybir.dt.bfloat16
F16 = mybir.dt.float16
F32 = mybir.dt.float32

B, T, C = 2, 2048, 1024
H, DK = 16, 64
P = 128
KC = C // P          # 8 contraction chunks over C
NTG = T // 512       # 4 t-groups of 512
NTJ = T // 128       # 16 t-chunks of 128
HPC = 4              # heads per core
VS = 66              # vaug per-head stride (col0 ones, 1-64 V, 65 pad)

NEG = -30000.0

# module-level knobs for test harness
TRACE = False
TRACE_KWARGS = {}
LAST_RESULTS = None


def _emit(ctx, tc, aps):
    nc = tc.nc
    xt, wq, wk, wv, bq, bk, wp, mask, out = (
        aps["xt"], aps["wq"], aps["wk"], aps["wv"], aps["bq"], aps["bk"],
        aps["wp"], aps["mask"], aps["out"],
    )

    consts = ctx.enter_context(tc.tile_pool(name="consts", bufs=1))
    bigs = ctx.enter_context(tc.tile_pool(name="bigs", bufs=1))
    temps = ctx.enter_context(tc.tile_pool(name="temps", bufs=4))
    ppool = ctx.enter_context(tc.tile_pool(name="ppool", bufs=4))
    psum = ctx.enter_context(tc.tile_pool(name="psum", bufs=1, space="PSUM"))
    dpool = ctx.enter_context(tc.tile_pool(name="dpool", bufs=2, space="DRAM"))

    # ---- load inputs to SBUF. All DRAM inputs are pre-shaped on the host so
    # every DMA reads fully-linear DRAM (weights first — the first matmuls
    # need them; xt arrives as per-t-group blocks [tg][p][k][512]) ----
    wqs = consts.tile([P, KC, 2 * P], BF)
    nc.sync.dma_start(out=wqs, in_=wq)
    wks = consts.tile([P, KC, 2 * P], BF)
    nc.gpsimd.dma_start(out=wks, in_=wk)

    xts = bigs.tile([P, KC, T], BF)
    # first two t-groups split by k on both queues so the first QKV
    # accumulation can stream chunks as they land
    for k0, k1 in ((0, 2), (2, 5), (5, 8)):
        nc.sync.dma_start(out=xts[:, k0:k1, 0:512], in_=xt[0, :, k0:k1, :])
        nc.gpsimd.dma_start(out=xts[:, k0:k1, 512:1024], in_=xt[1, :, k0:k1, :])

    bqs = consts.tile([P, 2], F32)
    nc.sync.dma_start(out=bqs, in_=bq.rearrange("(m p) -> p m", p=P))
    bks = consts.tile([P, 2], F32)
    nc.sync.dma_start(out=bks, in_=bk.rearrange("(m p) -> p m", p=P))
    maskt = consts.tile([P, P], F32)
    nc.sync.dma_start(out=maskt, in_=mask)
    wvs = consts.tile([P, KC, 2 * P], BF)
    nc.gpsimd.dma_start(out=wvs, in_=wv)
    nc.sync.dma_start(out=xts[:, :, ts(2, 512)], in_=xt[2])
    nc.gpsimd.dma_start(out=xts[:, :, ts(3, 512)], in_=xt[3])
    wps = consts.tile([P, 2, C], BF)
    nc.sync.dma_start(out=wps, in_=wp)

    # ---- Q^T / K^T: [128(d pair-packed), pair, T] ----
    qt = bigs.tile([P, 2, T], BF)
    kt = bigs.tile([P, 2, T], BF)

    def emit_qk_pair(m, pg):
        # one LDWEIGHTS per (k, dst) feeds two N=512 matmuls (t-groups
        # 2*pg and 2*pg+1) — the second, identical LDW is deduped later
        tga, tgb = 2 * pg, 2 * pg + 1
        for wsrc, bsrc, dst in ((wqs, bqs, qt), (wks, bks, kt)):
            pq2 = [
                psum.tile([P, 512], F32, tag="mm", bufs=2, name=f"pq{i}")
                for i in range(2)
            ]
            for k in range(KC):
                for i, tg in enumerate((tga, tgb)):
                    nc.tensor.matmul(
                        pq2[i],
                        lhsT=wsrc[:, k, ts(m, P)],
                        rhs=xts[:, k, ts(tg, 512)],
                        start=(k == 0),
                        stop=(k == KC - 1),
                    )
            for i, tg in enumerate((tga, tgb)):
                nc.vector.tensor_add(
                    out=dst[:, m, ts(tg, 512)],
                    in0=pq2[i],
                    in1=bsrc[:, m : m + 1].to_broadcast([P, 512]),
                )

    # ---- V -> vaug [128, tj, 4*66] (col DK = ones) ----
    vaug = bigs.tile([P, NTJ, HPC * VS], BF)
    vaug4 = vaug.rearrange("p t (h c) -> p t h c", c=VS)

    def emit_v(g):
        for tj in range(4 * g, 4 * g + 4):
            pv = psum.tile([P, 512], F32, tag="mm", bufs=2, name="pv")
            for k in range(KC):
                nc.tensor.matmul(
                    pv[:, : 2 * P],
                    lhsT=xts[:, k, ts(tj, P)],
                    rhs=wvs[:, k, :],
                    start=(k == 0),
                    stop=(k == KC - 1),
                )
            nc.vector.tensor_copy(
                out=vaug4[:, tj, :, 0:DK],
                in_=pv[:, : 2 * P].rearrange("p (h d) -> p h d", d=DK),
            )

    # ---- attention ----
    yts = [bigs.tile([P, T], BF, name=f"yt{m}") for m in range(2)]

    def emit_attn(m, g):
        po = [
            psum.tile([DK + 1, 512], F32, tag=f"o{h}", bufs=1, name=f"po{h}")
            for h in range(2)
        ]
        njc = 4 * g + 4
        for j in range(njc):
            jrel = j - 4 * g
            band = jrel >= 0
            ncols = 512 - 128 * jrel if band else 512
            qoff = g * 512 + (128 * jrel if band else 0)
            pss = []
            for h in range(2):
                ps = psum.tile([P, 512], F32, tag=f"s{h}", bufs=2, name=f"ps{h}")
                nc.tensor.matmul(
                    ps[:, :ncols],
                    lhsT=kt[h * DK : (h + 1) * DK, m, ts(j, P)],
                    rhs=qt[h * DK : (h + 1) * DK, m, ds(qoff, ncols)],
                    start=True,
                    stop=True,
                    tile_position=(h * DK, 0),
                )
                pss.append(ps)
            if band:
                for h in range(2):
                    nc.vector.tensor_add(
                        out=pss[h][:, :P], in0=pss[h][:, :P], in1=maskt
                    )
            for h in range(2):
                pt = ppool.tile([P, 512], BF, tag=f"p{h}", name=f"pt{h}")
                nc.scalar.activation(
                    pt[:, :ncols],
                    pss[h][:, :ncols],
                    mybir.ActivationFunctionType.Exp,
                )
                co = 128 * jrel if band else 0
                nc.tensor.matmul(
                    po[h][:, co : co + ncols],
                    lhsT=vaug4[:, j, 2 * m + h, : DK + 1],
                    rhs=pt[:, :ncols],
                    start=(j == 0),
                    stop=(j == njc - 1),
                    skip_group_check=True,
                )
        # finalize: copy O^T off PSUM fast, then normalize rows 0-63 by the
        # broadcast exp-sum (row 64) and place into yt
        for h in range(2):
            oc = temps.tile([P, 512], F32, tag="oc", name="oc")
            nc.vector.tensor_copy(out=oc[: DK + 1, :], in_=po[h])
            dscr = dpool.tile([512], F32, tag="dscr", name="dscr")
            nc.sync.dma_start(out=dscr, in_=oc[DK : DK + 1, :])
            rbl = temps.tile([P, 512], F32, tag="rbl", name="rbl")
            nc.gpsimd.dma_start(
                out=rbl[:DK, :],
                in_=bass.AP(
                    tensor=dscr.tensor,
                    offset=dscr.offset,
                    ap=[[0, DK]] + list(dscr.ap),
                ),
            )
            rb = temps.tile([P, 512], F32, tag="rb", name="rb")
            nc.vector.reciprocal_approx_fast(out=rb[:DK, :], in_=rbl[:DK, :])
            stg = temps.tile([P, 512], BF, tag="stg", name="stg")
            nc.vector.tensor_mul(
                out=stg[:DK, :],
                in0=oc[:DK, :],
                in1=rb[:DK, :],
            )
            nc.sync.dma_start(
                out=yts[m][h * DK : (h + 1) * DK, ts(g, 512)],
                in_=stg[:DK, :],
            )

    # ---- output projection: partial [T, C] for one t-group of 4 chunks ----
    def emit_proj(g):
        for tj in range(4 * g, 4 * g + 4):
            pps = [
                psum.tile([P, 512], F32, tag="mm", bufs=2, name=f"pp{n}")
                for n in range(2)
            ]
            for kc in range(2):
                for n in range(2):
                    nc.tensor.matmul(
                        pps[n],
                        lhsT=yts[kc][:, ts(tj, P)],
                        rhs=wps[:, kc, ts(n, 512)],
                        start=(kc == 0),
                        stop=(kc == 1),
                    )
            for n in range(2):
                ostg = temps.tile([P, 512], F16, tag="ostg", name="ostg")
                nc.vector.tensor_copy(out=ostg, in_=pps[n])
                eng = nc.sync if n == 0 else nc.gpsimd
                eng.dma_start(out=out[ts(tj, P), ts(n, 512)], in_=ostg)

    # ---- schedule: pipeline by q-group, weaving PE-dense QKV/proj work
    # between ACT-gated attention so both engine queues stay fed. Group
    # order [1,2,3,0] puts the smallest attention group (g=0) last so the
    # kernel tail is short. QKV/V tiles are emitted incrementally just
    # before the first group that needs them. ----
    nc.vector.memset(vaug4[:, :, :, DK : DK + 1], 1.0)
    order = [1, 2, 3, 0]
    qk_done = [0, 0]  # per head-pair: number of t-group PAIRS emitted
    v_done = 0
    proj_queue = []
    for g in order:
        need_pg = g // 2 + 1
        need_v = g + 1
        while qk_done[0] < need_pg:
            emit_qk_pair(0, qk_done[0])
            qk_done[0] += 1
        while v_done < need_v:
            emit_v(v_done)
            v_done += 1
        emit_attn(0, g)
        while qk_done[1] < need_pg:
            emit_qk_pair(1, qk_done[1])
            qk_done[1] += 1
        emit_attn(1, g)
        proj_queue.append(g)
        if len(proj_queue) > 1:
            emit_proj(proj_queue.pop(0))
    for g in proj_queue:
        emit_proj(g)


def _dedupe_ldweights(nc):
    """Drop an InstLdweights when the immediately-preceding PE weight load in
    the scheduled stream is byte-identical (only matmuls in between — they
    don't disturb the stationary operand). Saves ~100ns of serialized PE time
    per duplicate."""
    removed = 0
    for f in nc.m.functions:
        for bb in f.blocks:
            insts = list(bb.instructions)
            last_sig = None
            to_remove = []
            for inst in insts:
                tn = type(inst).__name__
                if tn == "InstLdweights":
                    si = inst.sync_info
                    has_sync = si is not None and (
                        list(si.on_wait) or list(si.on_update)
                    )
                    sig = (
                        str(inst.ins[0]),
                        str(inst.tile_position),
                        str(inst.tile_size),
                        str(inst.perf_mode),
                        str(inst.is_transpose),
                    )
                    if sig == last_sig and not has_sync:
                        to_remove.append(inst)
                        continue
                    last_sig = sig
                elif tn == "InstMatmult":
                    continue
                elif getattr(inst, "engine", None) == mybir.EngineType.PE:
                    last_sig = None
            for inst in to_remove:
                bb.instructions.remove(inst)
                removed += 1
    return removed


_NC_CACHE = None


def build():
    global _NC_CACHE
    if _NC_CACHE is not None:
        return _NC_CACHE
    nc = bacc.Bacc("TRN2", target_bir_lowering=False, debug=False, num_devices=8)
    aps = {
        "xt": nc.dram_tensor("xt", [NTG, P, KC, 512], BF, kind="ExternalInput").ap(),
        "wq": nc.dram_tensor("wq", [P, KC, 2 * P], BF, kind="ExternalInput").ap(),
        "wk": nc.dram_tensor("wk", [P, KC, 2 * P], BF, kind="ExternalInput").ap(),
        "wv": nc.dram_tensor("wv", [P, KC, 2 * P], BF, kind="ExternalInput").ap(),
        "bq": nc.dram_tensor("bq", [2 * P], F32, kind="ExternalInput").ap(),
        "bk": nc.dram_tensor("bk", [2 * P], F32, kind="ExternalInput").ap(),
        "wp": nc.dram_tensor("wp", [P, 2, C], BF, kind="ExternalInput").ap(),
        "mask": nc.dram_tensor("mask", [P, P], F32, kind="ExternalInput").ap(),
        "out": nc.dram_tensor("out", [T, C], F16, kind="ExternalOutput").ap(),
    }
    with tile.TileContext(nc) as tc:
        with ExitStack() as ctx:
            _emit(ctx, tc, aps)
    _dedupe_ldweights(nc)
    nc.compile()
    _NC_CACHE = nc
    return nc


def make_in_maps(x, Wqkv, bqkv, Wproj):
    """Host-side sharding/layout prep. Returns per-core input dicts."""
    bf = ml_dtypes.bfloat16
    scale = np.float32(1.0 / np.sqrt(DK))
    maskv = np.where(
        np.arange(P)[None, :] >= np.arange(P)[:, None], 0.0, NEG
    ).astype(np.float32)
    def lay_w(w):  # [C, n] -> [p, k, n] linear
        n = w.shape[1]
        return np.ascontiguousarray(
            w.reshape(KC, P, n).transpose(1, 0, 2)
        ).astype(bf)

    def lay_x(xb):  # [T, C] -> [tg, p, k, 512] linear
        xt = xb.T  # [C, T]
        return np.ascontiguousarray(
            xt.reshape(KC, P, NTG, 512).transpose(2, 1, 0, 3)
        ).astype(bf)

    xts = [lay_x(x[b]) for b in range(B)]
    in_maps = []
    for c in range(8):
        b, hg = divmod(c, 4)
        lo = hg * HPC * DK
        sl = slice(lo, lo + HPC * DK)
        in_maps.append(
            {
                "xt": xts[b],
                "wq": lay_w(Wqkv[:, 0 * C :][:, sl] * scale),
                "wk": lay_w(Wqkv[:, 1 * C :][:, sl]),
                "wv": lay_w(Wqkv[:, 2 * C :][:, sl]),
                "bq": np.ascontiguousarray(bqkv[0 * C :][sl] * scale).astype(np.float32),
                "bk": np.ascontiguousarray(bqkv[1 * C :][sl]).astype(np.float32),
                "wp": np.ascontiguousarray(
                    Wproj[sl, :].reshape(2, P, C).transpose(1, 0, 2)
                ).astype(bf),
                "mask": maskv,
            }
        )
    return in_maps


def gather(outs, bqkv, Wproj, bproj):
    """Sum per-core partials per batch; fold V-bias + proj-bias analytically."""
    bv = bqkv[2 * C :].astype(np.float32)
    bp_eff = (bproj.astype(np.float32) + bv @ Wproj.astype(np.float32)).astype(
        np.float32
    )
    y = np.empty((B, T, C), np.float32)
    for b in range(B):
        acc = outs[b * 4 + 0].astype(np.float32)
        for hg in range(1, 4):
            acc += outs[b * 4 + hg].astype(np.float32)
        y[b] = acc + bp_eff[None, :]
    return y


def kernel(x, Wqkv, bqkv, Wproj, bproj):
    global LAST_RESULTS
    x = np.asarray(x, dtype=np.float32)
    Wqkv = np.asarray(Wqkv, dtype=np.float32)
    bqkv = np.asarray(bqkv, dtype=np.float32)
    Wproj = np.asarray(Wproj, dtype=np.float32)
    bproj = np.asarray(bproj, dtype=np.float32)

    nc = build()
    in_maps = make_in_maps(x, Wqkv, bqkv, Wproj)
    try:
        res = bass_utils.run_bass_kernel_spmd(
            nc,
            in_maps,
            core_ids=list(range(8)),
            trace=TRACE,
            **TRACE_KWARGS,
        )
    except Exception:
        if not TRACE:
            raise
        import traceback

        traceback.print_exc()
        print("traced run failed; retrying without trace", file=sys.stderr)
        res = bass_utils.run_bass_kernel_spmd(nc, in_maps, core_ids=list(range(8)))
    LAST_RESULTS = res
    outs = [res.results[c]["out"] for c in range(8)]
    return gather(outs, bqkv, Wproj, bproj)

